# revision 33
# baseline (speedup 1.0000x reference)
"""AGATCellWithMLP Trainium2 kernel: 8-core data-parallel over batch B.

v3 design (fp8 DoubleRow attention + host-static hypernetwork):
 - comb/kq/wv weights are fp8e4 with a 2-plane layout so qk and V run as
   DoubleRow matmuls (plane1 carries the 129th channel row, zero-padded);
   the x8/x64 quant scales are unwound (1/512) in the psum->sbuf copies.
 - exp() writes fp8e4 directly; the adjacency mask is additive -60000
   bf16 applied between leaky and exp; the attention numerator/denominator
   accumulate as fp8 DoubleRow matmuls over key-tile pairs (4x PE rate).
 - leaky_relu splits across ACT (Prelu), Pool (STT) and DVE (cast+STT)
   with tunable unit counts; exp stays on ACT.
 - softmax 1/den broadcast rides a PE ones-matmul into a spare psum
   column range instead of gpsimd row DMAs.
 - hypernetwork r|u gates: bf16 z-trick as v2 (fp8 fails precision).
   Candidate gate: the static x-channel half is computed on HOST and
   DMA'd in; the dynamic r*h half packs 2 qv-planes per matmul, halving
   both the z-build DVE time and the matmul count.
"""

import sys

sys.path.insert(0, "/opt/trn_rl_repo")

from contextlib import ExitStack

import numpy as np
import ml_dtypes

import concourse.bass as bass
import concourse.bacc as bacc
import concourse.tile as tile
from concourse import mybir
from concourse import bass_isa
from concourse.bass_utils import run_bass_kernel_spmd
from concourse.masks import make_identity
from concourse.bass import ts

P = 128
B, N, D, H, QV = 8, 1024, 64, 4, 32
C = 2 * D + 1            # 129
KD = C // 8              # 16
NQ = 512                 # selected nodes (queries) per graph
F32 = mybir.dt.float32
BF16 = mybir.dt.bfloat16
FP8 = mybir.dt.float8e4
AX = mybir.AxisListType
ALU = mybir.AluOpType
ACTF = mybir.ActivationFunctionType
DR = mybir.MatmulPerfMode.DoubleRow

NT = N // P              # 8 key tiles
NTP = NT // 2            # 4 key tile-pairs
QSCL = 1.0 / 512.0       # unwind comb(x8) * weights(x64) fp8 scales
NEG = -60000.0           # additive mask (bf16-safe, kills exp)

# per-unit leaky_relu engine assignment (16 units = 2 pairs x 8 tiles).
# 'a' = ACT Prelu (mask-add on Pool), 'd' = DVE mask-add + DVE STT leaky.
# Sim cannot run Prelu -> test.py flips USE_LRELU[0]=False to force the
# DVE path everywhere.
USE_LRELU = [True]
LEAKY_PLAN = list("aaaaadaaaaaaadaa")
MASK_POOL = [1, 3, 6, 9, 11, 15]


def build_graph(hw_leaky=True):
    nc = bacc.Bacc()

    comb8_d = nc.declare_dram_parameter("comb8", [P, 2 * N], FP8, False)
    combT_d = nc.declare_dram_parameter("combT", [P, N], BF16, False)
    c128_d = nc.declare_dram_parameter("c128", [1, N], BF16, False)
    kqw8_d = nc.declare_dram_parameter("kqw8", [P, 2 * 256], FP8, False)
    bkq_d = nc.declare_dram_parameter("bkq", [KD, 2 * H], F32, False)
    wv18_d = nc.declare_dram_parameter("wv18", [P, 2 * 516], FP8, False)
    adjN_d = nc.declare_dram_parameter("adjN", [P, NT * NQ], BF16, False)
    w2a_d = nc.declare_dram_parameter("w2a", [P, C], BF16, False)
    w2b_d = nc.declare_dram_parameter("w2b", [1, C], BF16, False)
    bias_d = nc.declare_dram_parameter("biases", [P, 4], F32, False)
    biasL_d = nc.declare_dram_parameter("biasesL", [1, 4], F32, False)
    qvT_d = nc.declare_dram_parameter("qvT", [QV, NQ], BF16, False)
    qb_d = nc.declare_dram_parameter("qb", [P, QV * NQ], BF16, False)
    wzru_d = nc.declare_dram_parameter("wzru", [P, QV * P], BF16, False)
    # packed candidate weights: [128, 16*64], row p -> Wc[2dd + p//64, p%64, o]
    wzc2_d = nc.declare_dram_parameter("wzc2", [P, 16 * D], BF16, False)
    # packed candidate qv: qb2[p, dd*512+m] = qv[m, 2dd + p//64]
    qb2_d = nc.declare_dram_parameter("qb2", [P, 16 * NQ], BF16, False)
    # [32, 256]: cols 0:128 wzruL (r|u row-128 weights), 128:256 bru
    small_d = nc.declare_dram_parameter("smalls", [QV, 256], BF16, False)
    # host-computed static candidate preactivation [64, 512] f32
    statc_d = nc.declare_dram_parameter("statc", [D, NQ], F32, False)
    out_d = nc.declare_dram_parameter("out", [NQ, D], F32, True)

    with tile.TileContext(nc) as tc, ExitStack() as ctx:
        sing = ctx.enter_context(tc.tile_pool(name="sing", bufs=1))
        smp = ctx.enter_context(tc.tile_pool(name="smp", bufs=3))
        pep = ctx.enter_context(tc.tile_pool(name="pep", bufs=3))
        work = ctx.enter_context(tc.tile_pool(name="work", bufs=3))
        # PSUM budget (8 banks): psS 2x[128,1024] = 4, psY 2x[128,512] = 2,
        # psE 2x[2,512] = 2.
        psS = ctx.enter_context(tc.tile_pool(name="psS", bufs=2, space="PSUM"))
        psY = ctx.enter_context(tc.tile_pool(name="psY", bufs=2, space="PSUM"))
        psE = ctx.enter_context(tc.tile_pool(name="psE", bufs=2, space="PSUM"))

        identf = sing.tile([P, P], F32)
        make_identity(nc, identf[:])
        ones1 = sing.tile([1, P], BF16, name="ones1")
        nc.gpsimd.memset(ones1[:], 1.0)
        ones2b = sing.tile([2, 1], BF16, name="ones2b")
        nc.gpsimd.memset(ones2b[:], 1.0)

        # ---------------- input DMAs (rough use order) --------------------
        comb8 = sing.tile([P, 2, N], FP8)
        c8v = comb8_d[:, :].rearrange("p (a n) -> p a n", a=2)
        nc.sync.dma_start(comb8[:, :, 0:NQ], c8v[:, :, 0:NQ])
        nc.scalar.dma_start(comb8[:, :, NQ:N], c8v[:, :, NQ:N])
        wv18 = sing.tile([P, 2, 516], FP8)
        nc.sync.dma_start(wv18[:], wv18_d[:, :].rearrange(
            "p (a n) -> p a n", a=2))
        kqw8 = sing.tile([P, 2, 256], FP8)
        nc.sync.dma_start(kqw8[:], kqw8_d[:, :].rearrange(
            "p (a n) -> p a n", a=2))
        bkq = sing.tile([KD, 2 * H], F32)
        nc.sync.dma_start(bkq[:], bkq_d[:, :])
        combT = sing.tile([P, N], BF16)
        nc.sync.dma_start(combT[:], combT_d[:, :])
        cxr = sing.tile([1, N], BF16)            # channel-128 row (last x)
        nc.sync.dma_start(cxr[:], c128_d[:, :])
        adjN = sing.tile([P, NT * NQ], BF16)
        nc.sync.dma_start(adjN[:], adjN_d[:, :])
        w2a = sing.tile([P, C], BF16)
        w2b = sing.tile([1, C], BF16)
        nc.sync.dma_start(w2a[:], w2a_d[:, :])
        nc.sync.dma_start(w2b[:], w2b_d[:, :])
        biases = sing.tile([P, 4], F32)   # cols: 0 = b1, 1 = b2
        biasesL = sing.tile([1, 4], F32)
        nc.sync.dma_start(biases[:], bias_d[:, :])
        nc.sync.dma_start(biasesL[:], biasL_d[:, :])
        qvT = sing.tile([QV, NQ], BF16)
        nc.sync.dma_start(qvT[:], qvT_d[:, :])
        statc = sing.tile([D, NQ], F32)
        nc.sync.dma_start(statc[:], statc_d[:, :])
        # big hyper-stage prefetches issued after the V phase below
        qb = sing.tile([P, QV * NQ], BF16)
        wzru = sing.tile([P, QV * P], BF16)
        wzc2 = sing.tile([P, 16 * D], BF16)
        qb2 = sing.tile([P, 16 * NQ], BF16)
        smalls = sing.tile([QV, 256], BF16)

        # ---------------- V phase: U = comb @ (Wv W1), all heads ----------
        # vtp[tp] layout: [128, plane(2), head(4), 130]; col 128 = c128 row,
        # col 129 = ones (for the softmax denominator).
        vtp = [sing.tile([P, 2, H, 132], FP8, tag=f"vtp{i}", name=f"vtp{i}")
               for i in range(NTP)]
        for i in range(NTP):
            nc.gpsimd.memset(vtp[i][:, :, :, 130:132], 0.0)
        for i in range(NT):
            pv = psS.tile([P, N], F32, tag="ps", name="pv")
            for g in range(2):
                nc.tensor.matmul(pv[:, g * NQ:g * NQ + 258],
                                 comb8[:, :, ts(i, P)],
                                 wv18[:, :, g * 258:(g + 1) * 258],
                                 start=True, stop=True, perf_mode=DR)
            # cols 0:128 = channels, col 128 = ones (softmax denominator),
            # col 129 = c128 channel -> E psum rows land [den@p0, cnum@p1]
            pvv = (pv[:].rearrange("p (g b) -> p g b", b=NQ)[:, :, 0:258]
                   .rearrange("p g (hh c) -> p g hh c", c=129))
            if i % 2 == 0:
                nc.vector.tensor_scalar_mul(
                    vtp[i // 2][:, i % 2, :, 0:128]
                    .rearrange("p (g hh) c -> p g hh c", g=2),
                    pvv[:, :, :, 0:128], QSCL)
                nc.vector.tensor_scalar_mul(
                    vtp[i // 2][:, i % 2, :, 129:130]
                    .rearrange("p (g hh) c -> p g hh c", g=2),
                    pvv[:, :, :, 128:129], QSCL)
            else:
                nc.scalar.activation(
                    vtp[i // 2][:, i % 2, :, 0:128]
                    .rearrange("p (g hh) c -> p g hh c", g=2),
                    pvv[:, :, :, 0:128], ACTF.Identity, scale=QSCL)
                nc.scalar.activation(
                    vtp[i // 2][:, i % 2, :, 129:130]
                    .rearrange("p (g hh) c -> p g hh c", g=2),
                    pvv[:, :, :, 128:129], ACTF.Identity, scale=QSCL)
            nc.gpsimd.memset(vtp[i // 2][:, i % 2, :, 128:129], 1.0)

        # ---------------- qk: per head-pair packed DR matmul --------------
        # psum rows per pair: [k_h0(16)@0 .. q_h0(16)@32 .. k_h1@64 q_h1@96]
        kT = [sing.tile([KD, N], BF16, tag=f"kT{h}", name=f"kT{h}")
              for h in range(H)]
        qT = [sing.tile([KD, NQ], BF16, tag=f"qT{h}", name=f"qT{h}")
              for h in range(H)]
        for p_ in range(2):
            ps = psS.tile([P, N], F32, tag="ps", name="qk")
            for half in range(2):
                nc.tensor.matmul(ps[:, ts(half, NQ)],
                                 kqw8[:, :, ts(p_, P)],
                                 comb8[:, :, ts(half, NQ)],
                                 start=True, stop=True, perf_mode=DR)
            for hh in range(2):
                h = 2 * p_ + hh
                if hh == 0:
                    nc.vector.scalar_tensor_tensor(
                        kT[h][:], ps[64 * hh:64 * hh + KD, :], QSCL,
                        bkq[:, h:h + 1].to_broadcast((KD, N)),
                        op0=ALU.mult, op1=ALU.add)
                else:
                    nc.scalar.activation(kT[h][:], ps[64 * hh:64 * hh + KD, :],
                                         ACTF.Identity, bias=bkq[:, h:h + 1],
                                         scale=QSCL)
                nc.scalar.activation(qT[h][:],
                                     ps[64 * hh + 32:64 * hh + 48, 0:NQ],
                                     ACTF.Identity, bias=bkq[:, H + h:H + h + 1],
                                     scale=QSCL)

        nc.sync.dma_start(qb[:], qb_d[:, :])
        nc.sync.dma_start(wzru[:], wzru_d[:, :])
        nc.sync.dma_start(wzc2[:], wzc2_d[:, :])
        nc.sync.dma_start(qb2[:], qb2_d[:, :])
        nc.sync.dma_start(smalls[:], small_d[:, :])

        # ---------------- attention + per-pair softmax norm ---------------
        m1acc = sing.tile([P, NQ], F32, name="m1acc")
        crs = [sing.tile([2, NQ], F32, tag=f"crs{j}", name=f"crs{j}")
               for j in range(H)]
        unit = 0
        for p_ in range(2):
            Y = [psY.tile([P, NQ], F32, tag="Y", name=f"Y{hh}")
                 for hh in range(2)]
            E = [psE.tile([4, NQ], F32, tag="E", name=f"E{hh}")
                 for hh in range(2)]
            h0, h1 = 2 * p_, 2 * p_ + 1
            pes = {}

            def accum(tp):
                pe2 = pes.pop(tp)
                st, sp = tp == 0, tp == NTP - 1
                for hh in range(2):
                    h = 2 * p_ + hh
                    nc.tensor.matmul(E[hh][:], vtp[tp][:, :, h, 128:132],
                                     pe2[:, hh, :, :], start=st, stop=sp,
                                     perf_mode=DR)
                    nc.tensor.matmul(Y[hh][:], vtp[tp][:, :, h, 0:P],
                                     pe2[:, hh, :, :], start=st, stop=sp,
                                     perf_mode=DR)

            for i in range(NT):
                ps = psS.tile([P, N], F32, tag="ps", name="sc")
                nc.tensor.matmul(ps[:, 0:NQ], kT[h0][:, ts(i, P)], qT[h0][:],
                                 start=True, stop=True)
                nc.tensor.matmul(ps[:, NQ:N], kT[h1][:, ts(i, P)], qT[h1][:],
                                 start=True, stop=True)
                sm = smp.tile([P, N], BF16, tag="sm", name="sm")
                adjbc = adjN[:, None, ts(i, NQ)].to_broadcast((P, 2, NQ))
                if hw_leaky and LEAKY_PLAN[unit] == "a":
                    # leaky on ACT, additive mask on DVE (or Pool for some)
                    nc.scalar.activation(sm[:], ps[:], ACTF.Prelu, alpha=0.2)
                    eng = nc.gpsimd if unit in MASK_POOL else nc.vector
                    eng.tensor_tensor(
                        sm[:].rearrange("p (a b) -> p a b", b=NQ),
                        sm[:].rearrange("p (a b) -> p a b", b=NQ),
                        adjbc, ALU.add)
                else:
                    # additive mask rides the psum read, then DVE leaky
                    t02 = smp.tile([P, N], BF16, tag="sm", name="t02")
                    nc.vector.tensor_tensor(
                        t02[:].rearrange("p (a b) -> p a b", b=NQ),
                        ps[:].rearrange("p (a b) -> p a b", b=NQ),
                        adjbc, ALU.add)
                    nc.vector.scalar_tensor_tensor(
                        sm[:], t02[:], 0.2, t02[:], op0=ALU.mult, op1=ALU.max)
                unit += 1
                if i % 2 == 0:
                    pe2 = pep.tile([P, 2, 2, NQ], FP8, tag="pe", name="pe")
                    pes[i // 2] = pe2
                else:
                    pe2 = pes[i // 2]
                nc.scalar.activation(
                    pe2[:, :, i % 2, :],
                    sm[:].rearrange("p (a b) -> p a b", b=NQ), ACTF.Exp)
                # software pipeline: numerator matmuls run 1 tile-pair
                # behind the scores.
                if i >= 3 and i % 2 == 1:
                    accum(i // 2 - 1)
            accum(NTP - 1)
            # per-pair normalization (frees Y/E psums for the next pair).
            # E rows are [den@p0, cnum@p1]; recip reads partition 0 of psum.
            # Y/E copy out to SBUF; the rinv broadcast reuses the freed psY
            # banks so the rb product reads exactly one psum operand.
            rinv = sing.tile([1, N], F32, tag=f"ri{p_}", name=f"ri{p_}")
            rinvb = sing.tile([1, N], BF16, tag=f"rib{p_}", name=f"rib{p_}")
            ysb = [sing.tile([P, NQ], F32, tag=f"ysb{p_}{hh}",
                             name=f"ysb{p_}{hh}") for hh in range(2)]
            esb = [sing.tile([2, NQ], F32, tag=f"esb{p_}{hh}",
                             name=f"esb{p_}{hh}") for hh in range(2)]
            for hh in range(2):
                nc.vector.reciprocal_approx_fast(rinv[0:1, ts(hh, NQ)],
                                                 E[hh][0:1, :])
                if p_ == 0:
                    nc.scalar.activation(ysb[hh][:], Y[hh][:], ACTF.Identity)
                    nc.scalar.activation(esb[hh][:], E[hh][0:2, :],
                                         ACTF.Identity)
                else:
                    nc.vector.tensor_copy(ysb[hh][:], Y[hh][:])
                    nc.vector.tensor_copy(esb[hh][:], E[hh][0:2, :])
            nc.vector.tensor_copy(rinvb[:], rinv[:])
            rbY = [psY.tile([P, NQ], F32, tag="Y", name=f"rbY{hh}")
                   for hh in range(2)]
            for hh in range(2):
                nc.tensor.matmul(rbY[hh][:], ones1[:],
                                 rinvb[0:1, ts(hh, NQ)], start=True,
                                 stop=True)
            for hh in range(2):
                nc.vector.tensor_tensor(crs[2 * p_ + hh][:], esb[hh][:],
                                        rbY[hh][0:2, :], ALU.mult)
                if p_ == 0 and hh == 0:
                    nc.vector.tensor_tensor(m1acc[:], ysb[hh][:],
                                            rbY[hh][:], ALU.mult)
                else:
                    t_ = work.tile([P, NQ], F32, tag="nt", name="nt")
                    nc.vector.tensor_tensor(t_[:], ysb[hh][:],
                                            rbY[hh][:], ALU.mult)
                    eng = nc.gpsimd if p_ == 0 else nc.vector
                    eng.tensor_tensor(m1acc[:], m1acc[:], t_[:], ALU.add)

        # ---------------- MLP channel 128 + relu + W2 + residual ----------
        c4a = sing.tile([2, NQ], BF16, name="c4a")
        c4b = sing.tile([2, NQ], BF16, name="c4b")
        nc.vector.tensor_tensor(c4a[:], crs[0][:], crs[1][:], ALU.add)
        nc.vector.tensor_tensor(c4b[:], crs[2][:], crs[3][:], ALU.add)
        nc.vector.tensor_tensor(c4a[:], c4a[:], c4b[:], ALU.add)
        c4p = psE.tile([4, NQ], F32, tag="E", name="c4p")
        nc.tensor.matmul(c4p[0:1, :], ones2b[:], c4a[:], start=True,
                         stop=True)
        m1T = sing.tile([P, NQ], BF16, name="m1T")
        nc.scalar.activation(m1T[:], m1acc[:], ACTF.Relu, bias=biases[:, 0:1])
        m1L = sing.tile([1, NQ], BF16, name="m1L")
        # b1L has the 4.0 (sum of den*rinv over heads) pre-subtracted on host
        nc.scalar.activation(m1L[:], c4p[0:1, :], ACTF.Relu,
                             bias=biasesL[0:1, 0:1])
        pcf = psY.tile([P, NQ], F32, tag="Y", name="pcf")
        nc.tensor.matmul(pcf[:], w2a[:, 0:P], m1T[:], start=True, stop=False)
        nc.tensor.matmul(pcf[:], w2b[:, 0:P], m1L[:], start=False, stop=True)
        pcfL = psE.tile([2, NQ], F32, tag="E", name="pcfL")
        nc.tensor.matmul(pcfL[0:1, :], w2a[:, P:C], m1T[:], start=True,
                         stop=False)
        nc.tensor.matmul(pcfL[0:1, :], w2b[:, P:C], m1L[:], start=False,
                         stop=True)
        cf = sing.tile([P, NQ], BF16, name="cf")
        nc.vector.scalar_tensor_tensor(cf[:], pcf[:], biases[:, 1:2],
                                       combT[:, 0:NQ], op0=ALU.add,
                                       op1=ALU.add)
        cl_row = sing.tile([1, NQ], BF16, name="cl_row")
        nc.vector.scalar_tensor_tensor(cl_row[:], pcfL[0:1, :],
                                       biasesL[0:1, 1:2], cxr[0:1, 0:NQ],
                                       op0=ALU.add, op1=ALU.add)

        # ---------------- hypernetwork r|u (z-trick, transposed) ----------
        zq = [sing.tile([P, 4 * NQ], BF16, tag=f"zq{g}", name=f"zq{g}")
              for g in range(8)]

        # r | u: bias matmul first (no z dependency), then 32 d-planes,
        # then the c128-channel term via qcl.
        ones32 = sing.tile([1, QV], BF16, name="ones32")
        nc.gpsimd.memset(ones32[:], 1.0)
        repp = psS.tile([P, N], F32, tag="ps", name="repp")
        nc.tensor.matmul(repp[0:QV, 0:NQ], ones32[:], cl_row[:], start=True,
                         stop=True)
        for g in range(8):
            eng = nc.gpsimd if g >= 6 else nc.vector
            eng.tensor_tensor(
                zq[g][:].rearrange("p (a b) -> p a b", b=NQ),
                cf[:, None, 0:NQ].to_broadcast((P, 4, NQ)),
                qb[:, g * 2048:(g + 1) * 2048].rearrange(
                    "p (a b) -> p a b", b=NQ),
                ALU.mult)
        qcl = sing.tile([QV, NQ], BF16, name="qcl")
        nc.vector.tensor_tensor(qcl[:], qvT[:], repp[0:QV, 0:NQ], ALU.mult)
        pru = psY.tile([P, NQ], F32, tag="Y", name="pru")
        nc.tensor.matmul(pru[:], smalls[:, P:256], qvT[:], start=True,
                         stop=False)
        for d in range(QV):
            g, dd = d // 4, d % 4
            nc.tensor.matmul(pru[:], wzru[:, d * P:(d + 1) * P],
                             zq[g][:, dd * NQ:(dd + 1) * NQ],
                             start=False, stop=False)
        nc.tensor.matmul(pru[:], smalls[:, 0:P], qcl[:], start=False,
                         stop=True)
        ru = sing.tile([P, NQ], BF16, name="ru")
        nc.scalar.activation(ru[:], pru[:], ACTF.Sigmoid)

        # ---------------- candidate gate (packed dynamic + host static) ---
        # selc2: rows 0:64 = r*hn, rows 64:128 = copy (for 2-plane packing)
        selc2 = sing.tile([P, NQ], BF16, name="selc2")
        nc.vector.tensor_tensor(selc2[0:D, :], ru[0:D, :], combT[0:D, 0:NQ],
                                ALU.mult)
        nc.vector.tensor_copy(selc2[D:P, :], selc2[0:D, :])
        u64 = sing.tile([D, NQ], BF16, name="u64")
        nc.scalar.copy(u64[:], ru[D:P, :])
        for g in range(4):
            eng = nc.gpsimd if g >= 3 else nc.vector
            eng.tensor_tensor(
                zq[g][:].rearrange("p (a b) -> p a b", b=NQ),
                selc2[:, None, 0:NQ].to_broadcast((P, 4, NQ)),
                qb2[:, g * 2048:(g + 1) * 2048].rearrange(
                    "p (a b) -> p a b", b=NQ),
                ALU.mult)
        pc = psY.tile([P, NQ], F32, tag="Y", name="pc")
        for dd in range(16):
            g, di = dd // 4, dd % 4
            nc.tensor.matmul(pc[0:D, :], wzc2[:, dd * D:(dd + 1) * D],
                             zq[g][:, di * NQ:(di + 1) * NQ],
                             start=dd == 0, stop=dd == 15)
        pcs = sing.tile([D, NQ], BF16, name="pcs")
        nc.vector.tensor_tensor(pcs[:], pc[0:D, :], statc[:], ALU.add)
        cand = sing.tile([D, NQ], BF16, name="cand")
        nc.scalar.activation(cand[:], pcs[:], ACTF.Tanh)

        # out = hn + u*(cand - hn)   (hn = selc2 rows 0:64, u = ru 64:128)
        d1 = sing.tile([D, NQ], BF16, name="d1")
        nc.vector.tensor_tensor(d1[:], cand[:], selc2[0:D, :], ALU.subtract)
        nc.vector.tensor_tensor(d1[:], d1[:], u64[:], ALU.mult)
        outT = sing.tile([D, NQ], F32, name="outT")
        for j in range(4):
            nc.vector.tensor_tensor(outT[:, ts(j, P)], d1[:, ts(j, P)],
                                    selc2[0:D, ts(j, P)], ALU.add)
            pt = psE.tile([P, D], F32, tag="E", name="pt")
            nc.tensor.transpose(pt[:, 0:D], outT[:, ts(j, P)],
                                identf[0:D, 0:D])
            ob = work.tile([P, D], F32, tag="ob", name="ob")
            nc.vector.tensor_copy(ob[:], pt[:, 0:D])
            dq = [nc.sync, nc.scalar, nc.gpsimd, nc.sync][j]
            dq.dma_start(out_d[ts(j, P), :], ob[:])

    return nc


_NC_CACHE = {}


def _get_nc():
    key = bool(USE_LRELU[0])
    if key not in _NC_CACHE:
        nc = build_graph(hw_leaky=key)
        if not nc.is_finalized():
            nc.finalize()
        _NC_CACHE[key] = nc
    return _NC_CACHE[key]


# channel reorder: new order = [h (64) | x (65)]
_R = np.concatenate([np.arange(65, 129), np.arange(0, 65)])
_BF = ml_dtypes.bfloat16
_E4 = ml_dtypes.float8_e4m3


def _bf(a):
    return np.ascontiguousarray(np.asarray(a, np.float32).astype(_BF))


def _f8(a, scale):
    x = np.clip(np.asarray(a, np.float32) * scale, -240, 240)
    return np.ascontiguousarray(x.astype(_E4))


def _prep_shared(Wq, bq, Wk, bk, Wv, bv, W1, b1, W2, b2, Wr, br, Wu, bu,
                 Wc, bc):
    f32 = np.float32
    Wq, bq = np.asarray(Wq, f32), np.asarray(bq, f32)
    Wk, bk = np.asarray(Wk, f32), np.asarray(bk, f32)
    Wv, bv = np.asarray(Wv, f32), np.asarray(bv, f32)
    W1, b1 = np.asarray(W1, f32).reshape(H, C, C), np.asarray(b1, f32)
    W2, b2 = np.asarray(W2, f32), np.asarray(b2, f32)

    # qk packed: per pair cols [k_h0|0|q_h0/4|0|k_h1|0|q_h1/4|0] (16 each);
    # biases ride the psum->sbuf copies as per-partition bias APs (bkq).
    Wq_r, Wk_r = Wq[:, _R, :], Wk[:, _R, :]
    kqw = np.zeros((C, 256), f32)
    bkq = np.zeros((KD, 2 * H), f32)
    for h in range(H):
        base = (h // 2) * 128 + (h % 2) * 64
        kqw[0:129, base:base + 16] = Wk_r[h]
        kqw[0:129, base + 32:base + 48] = Wq_r[h] * 0.25
        bkq[:, h] = bk[h]
        bkq[:, H + h] = bq[h] * 0.25
    kqw8 = np.zeros((P, 2, 256), f32)
    kqw8[:, 0, :] = kqw[0:128]
    kqw8[0, 1, :] = kqw[128]

    # V with W1 folded: U_h = Wv_h @ W1_h, contraction rows reordered
    U = np.stack([(Wv[h] @ W1[h])[_R] for h in range(H)])    # [H, 129, 129]
    wv1 = np.ascontiguousarray(
        np.transpose(U, (1, 0, 2)).reshape(C, H * C))        # [129, 516]
    b1_eff = b1 + sum(bv[h] @ W1[h] for h in range(H))
    wv18 = np.zeros((P, 2, 516), f32)
    wv18[:, 0, :] = wv1[0:128]
    wv18[0, 1, :] = wv1[128]

    w2r = W2[:, _R]                                          # cols reordered
    b2r = b2[_R]
    biases = np.zeros((C, 4), f32)
    biases[:, 0] = b1_eff
    biases[:, 1] = b2r
    # the c4 partition_all_reduce folds in sum_h den*rinv ~= H
    biases[128, 0] -= H

    Wr_r = np.asarray(Wr, f32)[:, _R, :]
    Wu_r = np.asarray(Wu, f32)[:, _R, :]
    Wc_r = np.asarray(Wc, f32)[:, _R, :]
    wzru = np.ascontiguousarray(np.transpose(
        np.concatenate([Wr_r[:, 0:128, :], Wu_r[:, 0:128, :]], 2),
        (1, 0, 2)).reshape(P, QV * P))
    # packed candidate weights [128, 16*64]
    wzc2 = np.zeros((P, 16, D), f32)
    for dd in range(16):
        wzc2[0:D, dd] = Wc_r[2 * dd, 0:D, :]
        wzc2[D:P, dd] = Wc_r[2 * dd + 1, 0:D, :]
    wzc2 = np.ascontiguousarray(wzc2.reshape(P, 16 * D))
    smalls = np.zeros((QV, 256), f32)
    smalls[:, 0:64] = Wr_r[:, 128, :]
    smalls[:, 64:128] = Wu_r[:, 128, :]
    smalls[:, 128:192] = np.asarray(br, f32)
    smalls[:, 192:256] = np.asarray(bu, f32)

    return dict(
        kqw8=_f8(kqw8.reshape(P, 512), 64.0),
        bkq=np.ascontiguousarray(bkq),
        wv18=_f8(wv18.reshape(P, 1032), 64.0),
        w2a=_bf(w2r[0:128]), w2b=_bf(w2r[128:129]),
        biases=np.ascontiguousarray(biases[0:128]),
        biasesL=np.ascontiguousarray(biases[128:129]),
        wzru=_bf(wzru), wzc2=_bf(wzc2), smalls=_bf(smalls),
        _Wc_r=Wc_r, _bc=np.asarray(bc, f32),
    )


def _prep_core(b, x, h, query_vectors, adj, nodes_n, shared):
    f32 = np.float32
    idx = nodes_n[b * NQ:(b + 1) * NQ].astype(np.int64)
    rest = np.setdiff1d(np.arange(N, dtype=np.int64), idx)
    perm = np.concatenate([idx, rest])
    comb = np.concatenate([x[b][perm], h[b][perm]], 1)[:, _R]  # [N,129] reord
    combT = np.ascontiguousarray(comb.T.astype(_BF))           # [129, N]
    comb8 = np.zeros((P, 2, N), f32)
    comb8[:, 0, :] = combT[0:128].astype(f32)
    comb8[0, 1, :] = combT[128].astype(f32)
    qv = query_vectors[b * NQ:(b + 1) * NQ]                    # [512, 32]
    qvT = np.ascontiguousarray(qv.T.astype(_BF))               # [32, 512]
    # qb [128, 32*512]: qb[p, d*512 + m] = qv[m, d]  (uint16 view = fast)
    qvT_u16 = qvT.view(np.uint16)
    qb = np.ascontiguousarray(
        np.broadcast_to(qvT_u16[None, :, :], (P, QV, NQ))
        .reshape(P, QV * NQ)).view(_BF)
    # qb2 [128, 16*512]: qb2[p, dd*512+m] = qv[m, 2dd + p//64]
    qb2 = np.empty((2, 16, NQ), np.uint16)
    for pl in range(2):
        qb2[pl] = qvT_u16[pl::2, :]
    qb2 = np.ascontiguousarray(
        np.broadcast_to(qb2[:, None, :, :], (2, D, 16, NQ))
        .reshape(P, 16 * NQ)).view(_BF)
    adjneg = np.where(adj[np.ix_(idx, perm)] != 0, 0.0, NEG).astype(f32).T
    adjN = np.ascontiguousarray(
        adjneg.reshape(NT, P, NQ).transpose(1, 0, 2).reshape(P, NT * NQ)
    ).astype(_BF)
    # host-computed static candidate preactivation:
    #   statc[o,m] = sum_{i in 64:129, d} Wc_r[d? ...] -- uses x rows + c128
    Wc_r, bc = shared["_Wc_r"], shared["_bc"]
    qvf = qvT.astype(f32)                                      # [32, 512]
    selS = combT[D:C, :NQ].astype(f32)                         # [65, 512]
    zS = (selS[:, None, :] * qvf[None, :, :]).reshape(65 * QV, NQ)
    WcS = Wc_r[:, D:C, :].transpose(1, 0, 2).reshape(65 * QV, D)
    statc = WcS.T @ zS + bc.T @ qvf                            # [64, 512]
    d = dict((k, v) for k, v in shared.items() if not k.startswith("_"))
    d["comb8"] = _f8(comb8.reshape(P, 2 * N), 8.0)
    d["combT"] = np.ascontiguousarray(combT[0:128])
    d["c128"] = np.ascontiguousarray(combT[128:129])
    d["qvT"] = qvT
    d["qb"] = qb
    d["qb2"] = qb2
    d["adjN"] = adjN
    d["statc"] = np.ascontiguousarray(statc.astype(f32))
    return d


def make_in_maps(x, h, query_vectors, adj, nodes_b, nodes_n, **weights):
    x = np.asarray(x, np.float32)
    h = np.asarray(h, np.float32)
    query_vectors = np.asarray(query_vectors, np.float32)
    adj = np.asarray(adj)
    nodes_n = np.asarray(nodes_n)
    shared = _prep_shared(**weights)
    return [_prep_core(b, x, h, query_vectors, adj, nodes_n, shared)
            for b in range(B)]


def kernel(x, h, query_vectors, adj, nodes_b, nodes_n,
           Wq, bq, Wk, bk, Wv, bv, W1, b1, W2, b2,
           Wr, br, Wu, bu, Wc, bc):
    in_maps = make_in_maps(
        x, h, query_vectors, adj, nodes_b, nodes_n,
        Wq=Wq, bq=bq, Wk=Wk, bk=bk, Wv=Wv, bv=bv, W1=W1, b1=b1, W2=W2, b2=b2,
        Wr=Wr, br=br, Wu=Wu, bu=bu, Wc=Wc, bc=bc)
    nc = _get_nc()
    res = run_bass_kernel_spmd(nc, in_maps, list(range(B)))
    outs = [np.asarray(res.results[b]["out"], np.float32) for b in range(B)]
    return np.concatenate(outs, axis=0)


# revision 34
# speedup vs baseline: 1.1079x; 1.1079x over previous
"""AGATCellWithMLP Trainium2 kernel: 8-core data-parallel over batch B.

v3 design (fp8 DoubleRow attention + host-static hypernetwork):
 - comb/kq/wv weights are fp8e4 with a 2-plane layout so qk and V run as
   DoubleRow matmuls (plane1 carries the 129th channel row, zero-padded);
   the x8/x64 quant scales are unwound (1/512) in the psum->sbuf copies.
 - exp() writes fp8e4 directly; the adjacency mask is additive -60000
   bf16 applied between leaky and exp; the attention numerator/denominator
   accumulate as fp8 DoubleRow matmuls over key-tile pairs (4x PE rate).
 - leaky_relu splits across ACT (Prelu), Pool (STT) and DVE (cast+STT)
   with tunable unit counts; exp stays on ACT.
 - softmax 1/den broadcast rides a PE ones-matmul into a spare psum
   column range instead of gpsimd row DMAs.
 - hypernetwork r|u gates: bf16 z-trick as v2 (fp8 fails precision).
   Candidate gate: the static x-channel half is computed on HOST and
   DMA'd in; the dynamic r*h half packs 2 qv-planes per matmul, halving
   both the z-build DVE time and the matmul count.
"""

import sys

sys.path.insert(0, "/opt/trn_rl_repo")

from contextlib import ExitStack

import numpy as np
import ml_dtypes

import concourse.bass as bass
import concourse.bacc as bacc
import concourse.tile as tile
from concourse import mybir
from concourse import bass_isa
from concourse.bass_utils import run_bass_kernel_spmd
from concourse.masks import make_identity
from concourse.bass import ts

P = 128
B, N, D, H, QV = 8, 1024, 64, 4, 32
C = 2 * D + 1            # 129
KD = C // 8              # 16
NQ = 512                 # selected nodes (queries) per graph
F32 = mybir.dt.float32
BF16 = mybir.dt.bfloat16
FP8 = mybir.dt.float8e4
AX = mybir.AxisListType
ALU = mybir.AluOpType
ACTF = mybir.ActivationFunctionType
DR = mybir.MatmulPerfMode.DoubleRow

NT = N // P              # 8 key tiles
NTP = NT // 2            # 4 key tile-pairs
QSCL = 1.0 / 512.0       # unwind comb(x8) * weights(x64) fp8 scales
NEG = -60000.0           # additive mask (bf16-safe, kills exp)

# per-unit leaky_relu engine assignment (16 units = 2 pairs x 8 tiles).
# 'a' = ACT Prelu (mask-add on Pool), 'd' = DVE mask-add + DVE STT leaky.
# Sim cannot run Prelu -> test.py flips USE_LRELU[0]=False to force the
# DVE path everywhere.
USE_LRELU = [True]
LEAKY_PLAN = list("aaaaadaaaaaaadaa")
MASK_POOL = [1, 3, 6, 9, 11, 15]


def build_graph(hw_leaky=True):
    nc = bacc.Bacc()

    comb8_d = nc.declare_dram_parameter("comb8", [P, 2 * N], FP8, False)
    combT_d = nc.declare_dram_parameter("combT", [P, N], BF16, False)
    c128_d = nc.declare_dram_parameter("c128", [1, N], BF16, False)
    kqw8_d = nc.declare_dram_parameter("kqw8", [P, 2 * 256], FP8, False)
    bkq_d = nc.declare_dram_parameter("bkq", [KD, 2 * H], F32, False)
    wv18_d = nc.declare_dram_parameter("wv18", [P, 2 * 516], FP8, False)
    adjN_d = nc.declare_dram_parameter("adjN", [P, NT * NQ], BF16, False)
    w2a_d = nc.declare_dram_parameter("w2a", [P, C], BF16, False)
    w2b_d = nc.declare_dram_parameter("w2b", [1, C], BF16, False)
    bias_d = nc.declare_dram_parameter("biases", [P, 4], F32, False)
    biasL_d = nc.declare_dram_parameter("biasesL", [1, 4], F32, False)
    qvT_d = nc.declare_dram_parameter("qvT", [QV, NQ], BF16, False)
    qb_d = nc.declare_dram_parameter("qb", [P, QV * NQ], BF16, False)
    wzru_d = nc.declare_dram_parameter("wzru", [P, QV * P], BF16, False)
    # packed candidate weights: [128, 16*64], row p -> Wc[2dd + p//64, p%64, o]
    wzc2_d = nc.declare_dram_parameter("wzc2", [P, 16 * D], BF16, False)
    # packed candidate qv: qb2[p, dd*512+m] = qv[m, 2dd + p//64]
    qb2_d = nc.declare_dram_parameter("qb2", [P, 16 * NQ], BF16, False)
    # [32, 256]: cols 0:128 wzruL (r|u row-128 weights), 128:256 bru
    small_d = nc.declare_dram_parameter("smalls", [QV, 256], BF16, False)
    # host-computed static candidate preactivation [64, 512] f32
    statc_d = nc.declare_dram_parameter("statc", [D, NQ], F32, False)
    out_d = nc.declare_dram_parameter("out", [NQ, D], F32, True)

    with tile.TileContext(nc) as tc, ExitStack() as ctx:
        sing = ctx.enter_context(tc.tile_pool(name="sing", bufs=1))
        smp = ctx.enter_context(tc.tile_pool(name="smp", bufs=3))
        pep = ctx.enter_context(tc.tile_pool(name="pep", bufs=3))
        work = ctx.enter_context(tc.tile_pool(name="work", bufs=3))
        # PSUM budget (8 banks): psS 2x[128,1024] = 4, psY 2x[128,512] = 2,
        # psE 2x[2,512] = 2.
        psS = ctx.enter_context(tc.tile_pool(name="psS", bufs=2, space="PSUM"))
        psY = ctx.enter_context(tc.tile_pool(name="psY", bufs=2, space="PSUM"))
        psE = ctx.enter_context(tc.tile_pool(name="psE", bufs=2, space="PSUM"))

        identf = sing.tile([P, P], F32)
        make_identity(nc, identf[:])
        ones1 = sing.tile([1, P], BF16, name="ones1")
        nc.gpsimd.memset(ones1[:], 1.0)
        ones2b = sing.tile([2, 1], BF16, name="ones2b")
        nc.gpsimd.memset(ones2b[:], 1.0)

        # ---------------- input DMAs (rough use order) --------------------
        comb8 = sing.tile([P, 2, N], FP8)
        nc.sync.dma_start(comb8[:], comb8_d[:, :].rearrange(
            "p (a n) -> p a n", a=2))
        wv18 = sing.tile([P, 2, 516], FP8)
        nc.sync.dma_start(wv18[:], wv18_d[:, :].rearrange(
            "p (a n) -> p a n", a=2))
        kqw8 = sing.tile([P, 2, 256], FP8)
        nc.sync.dma_start(kqw8[:], kqw8_d[:, :].rearrange(
            "p (a n) -> p a n", a=2))
        bkq = sing.tile([KD, 2 * H], F32)
        nc.sync.dma_start(bkq[:], bkq_d[:, :])
        combT = sing.tile([P, N], BF16)
        nc.sync.dma_start(combT[:], combT_d[:, :])
        cxr = sing.tile([1, N], BF16)            # channel-128 row (last x)
        nc.sync.dma_start(cxr[:], c128_d[:, :])
        adjN = sing.tile([P, NT * NQ], BF16)
        nc.sync.dma_start(adjN[:], adjN_d[:, :])
        w2a = sing.tile([P, C], BF16)
        w2b = sing.tile([1, C], BF16)
        nc.sync.dma_start(w2a[:], w2a_d[:, :])
        nc.sync.dma_start(w2b[:], w2b_d[:, :])
        biases = sing.tile([P, 4], F32)   # cols: 0 = b1, 1 = b2
        biasesL = sing.tile([1, 4], F32)
        nc.sync.dma_start(biases[:], bias_d[:, :])
        nc.sync.dma_start(biasesL[:], biasL_d[:, :])
        qvT = sing.tile([QV, NQ], BF16)
        nc.sync.dma_start(qvT[:], qvT_d[:, :])
        statc = sing.tile([D, NQ], F32)
        nc.sync.dma_start(statc[:], statc_d[:, :])
        # big hyper-stage prefetches issued after the V phase below
        qb = sing.tile([P, QV * NQ], BF16)
        wzru = sing.tile([P, QV * P], BF16)
        wzc2 = sing.tile([P, 16 * D], BF16)
        qb2 = sing.tile([P, 16 * NQ], BF16)
        smalls = sing.tile([QV, 256], BF16)

        # ---------------- V phase: U = comb @ (Wv W1), all heads ----------
        # vtp[tp] layout: [128, plane(2), head(4), 130]; col 128 = c128 row,
        # col 129 = ones (for the softmax denominator).
        vtp = [sing.tile([P, 2, H, 132], FP8, tag=f"vtp{i}", name=f"vtp{i}")
               for i in range(NTP)]
        for i in range(NTP):
            nc.gpsimd.memset(vtp[i][:, :, :, 130:132], 0.0)
        for i in range(NT):
            pv = psS.tile([P, N], F32, tag="ps", name="pv")
            for g in range(2):
                nc.tensor.matmul(pv[:, g * NQ:g * NQ + 258],
                                 comb8[:, :, ts(i, P)],
                                 wv18[:, :, g * 258:(g + 1) * 258],
                                 start=True, stop=True, perf_mode=DR)
            # cols 0:128 = channels, col 128 = ones (softmax denominator),
            # col 129 = c128 channel -> E psum rows land [den@p0, cnum@p1]
            pvv = (pv[:].rearrange("p (g b) -> p g b", b=NQ)[:, :, 0:258]
                   .rearrange("p g (hh c) -> p g hh c", c=129))
            if i % 2 == 0:
                nc.vector.tensor_scalar_mul(
                    vtp[i // 2][:, i % 2, :, 0:128]
                    .rearrange("p (g hh) c -> p g hh c", g=2),
                    pvv[:, :, :, 0:128], QSCL)
                nc.vector.tensor_scalar_mul(
                    vtp[i // 2][:, i % 2, :, 129:130]
                    .rearrange("p (g hh) c -> p g hh c", g=2),
                    pvv[:, :, :, 128:129], QSCL)
            else:
                nc.scalar.activation(
                    vtp[i // 2][:, i % 2, :, 0:128]
                    .rearrange("p (g hh) c -> p g hh c", g=2),
                    pvv[:, :, :, 0:128], ACTF.Identity, scale=QSCL)
                nc.scalar.activation(
                    vtp[i // 2][:, i % 2, :, 129:130]
                    .rearrange("p (g hh) c -> p g hh c", g=2),
                    pvv[:, :, :, 128:129], ACTF.Identity, scale=QSCL)
            nc.gpsimd.memset(vtp[i // 2][:, i % 2, :, 128:129], 1.0)

        # ---------------- qk: per head-pair packed DR matmul --------------
        # psum rows per pair: [k_h0(16)@0 .. q_h0(16)@32 .. k_h1@64 q_h1@96]
        kT = [sing.tile([KD, N], BF16, tag=f"kT{h}", name=f"kT{h}")
              for h in range(H)]
        qT = [sing.tile([KD, NQ], BF16, tag=f"qT{h}", name=f"qT{h}")
              for h in range(H)]
        for p_ in range(2):
            ps = psS.tile([P, N], F32, tag="ps", name="qk")
            for half in range(2):
                nc.tensor.matmul(ps[:, ts(half, NQ)],
                                 kqw8[:, :, ts(p_, P)],
                                 comb8[:, :, ts(half, NQ)],
                                 start=True, stop=True, perf_mode=DR)
            for hh in range(2):
                h = 2 * p_ + hh
                if hh == 0:
                    nc.vector.scalar_tensor_tensor(
                        kT[h][:], ps[64 * hh:64 * hh + KD, :], QSCL,
                        bkq[:, h:h + 1].to_broadcast((KD, N)),
                        op0=ALU.mult, op1=ALU.add)
                else:
                    nc.scalar.activation(kT[h][:], ps[64 * hh:64 * hh + KD, :],
                                         ACTF.Identity, bias=bkq[:, h:h + 1],
                                         scale=QSCL)
                nc.scalar.activation(qT[h][:],
                                     ps[64 * hh + 32:64 * hh + 48, 0:NQ],
                                     ACTF.Identity, bias=bkq[:, H + h:H + h + 1],
                                     scale=QSCL)

        nc.sync.dma_start(qb[:], qb_d[:, :])
        nc.sync.dma_start(wzru[:], wzru_d[:, :])
        nc.sync.dma_start(wzc2[:], wzc2_d[:, :])
        nc.sync.dma_start(qb2[:], qb2_d[:, :])
        nc.sync.dma_start(smalls[:], small_d[:, :])

        # ---------------- attention + per-pair softmax norm ---------------
        m1acc = sing.tile([P, NQ], F32, name="m1acc")
        crs = [sing.tile([2, NQ], F32, tag=f"crs{j}", name=f"crs{j}")
               for j in range(H)]
        unit = 0
        for p_ in range(2):
            Y = [psY.tile([P, NQ], F32, tag="Y", name=f"Y{hh}")
                 for hh in range(2)]
            E = [psE.tile([4, NQ], F32, tag="E", name=f"E{hh}")
                 for hh in range(2)]
            h0, h1 = 2 * p_, 2 * p_ + 1
            pes = {}

            def accum(tp):
                pe2 = pes.pop(tp)
                st, sp = tp == 0, tp == NTP - 1
                for hh in range(2):
                    h = 2 * p_ + hh
                    nc.tensor.matmul(E[hh][:], vtp[tp][:, :, h, 128:132],
                                     pe2[:, hh, :, :], start=st, stop=sp,
                                     perf_mode=DR)
                    nc.tensor.matmul(Y[hh][:], vtp[tp][:, :, h, 0:P],
                                     pe2[:, hh, :, :], start=st, stop=sp,
                                     perf_mode=DR)

            for i in range(NT):
                ps = psS.tile([P, N], F32, tag="ps", name="sc")
                nc.tensor.matmul(ps[:, 0:NQ], kT[h0][:, ts(i, P)], qT[h0][:],
                                 start=True, stop=True)
                nc.tensor.matmul(ps[:, NQ:N], kT[h1][:, ts(i, P)], qT[h1][:],
                                 start=True, stop=True)
                sm = smp.tile([P, N], BF16, tag="sm", name="sm")
                adjbc = adjN[:, None, ts(i, NQ)].to_broadcast((P, 2, NQ))
                if hw_leaky and LEAKY_PLAN[unit] == "a":
                    # leaky on ACT, additive mask on DVE (or Pool for some)
                    nc.scalar.activation(sm[:], ps[:], ACTF.Prelu, alpha=0.2)
                    eng = nc.gpsimd if unit in MASK_POOL else nc.vector
                    eng.tensor_tensor(
                        sm[:].rearrange("p (a b) -> p a b", b=NQ),
                        sm[:].rearrange("p (a b) -> p a b", b=NQ),
                        adjbc, ALU.add)
                else:
                    # additive mask rides the psum read, then DVE leaky
                    t02 = smp.tile([P, N], BF16, tag="sm", name="t02")
                    nc.vector.tensor_tensor(
                        t02[:].rearrange("p (a b) -> p a b", b=NQ),
                        ps[:].rearrange("p (a b) -> p a b", b=NQ),
                        adjbc, ALU.add)
                    nc.vector.scalar_tensor_tensor(
                        sm[:], t02[:], 0.2, t02[:], op0=ALU.mult, op1=ALU.max)
                unit += 1
                if i % 2 == 0:
                    pe2 = pep.tile([P, 2, 2, NQ], FP8, tag="pe", name="pe")
                    pes[i // 2] = pe2
                else:
                    pe2 = pes[i // 2]
                nc.scalar.activation(
                    pe2[:, :, i % 2, :],
                    sm[:].rearrange("p (a b) -> p a b", b=NQ), ACTF.Exp)
                # software pipeline: numerator matmuls run 1 tile-pair
                # behind the scores.
                if i >= 3 and i % 2 == 1:
                    accum(i // 2 - 1)
            accum(NTP - 1)
            # per-pair normalization (frees Y/E psums for the next pair).
            # E rows are [den@p0, cnum@p1]; recip reads partition 0 of psum.
            # Y/E copy out to SBUF; the rinv broadcast reuses the freed psY
            # banks so the rb product reads exactly one psum operand.
            rinv = sing.tile([1, N], F32, tag=f"ri{p_}", name=f"ri{p_}")
            rinvb = sing.tile([1, N], BF16, tag=f"rib{p_}", name=f"rib{p_}")
            ysb = [sing.tile([P, NQ], F32, tag=f"ysb{p_}{hh}",
                             name=f"ysb{p_}{hh}") for hh in range(2)]
            esb = [sing.tile([2, NQ], F32, tag=f"esb{p_}{hh}",
                             name=f"esb{p_}{hh}") for hh in range(2)]
            for hh in range(2):
                nc.vector.reciprocal_approx_fast(rinv[0:1, ts(hh, NQ)],
                                                 E[hh][0:1, :])
                if p_ == 0:
                    nc.scalar.activation(ysb[hh][:], Y[hh][:], ACTF.Identity)
                    nc.scalar.activation(esb[hh][:], E[hh][0:2, :],
                                         ACTF.Identity)
                else:
                    nc.vector.tensor_copy(ysb[hh][:], Y[hh][:])
                    nc.vector.tensor_copy(esb[hh][:], E[hh][0:2, :])
            nc.vector.tensor_copy(rinvb[:], rinv[:])
            rbY = [psY.tile([P, NQ], F32, tag="Y", name=f"rbY{hh}")
                   for hh in range(2)]
            for hh in range(2):
                nc.tensor.matmul(rbY[hh][:], ones1[:],
                                 rinvb[0:1, ts(hh, NQ)], start=True,
                                 stop=True)
            for hh in range(2):
                nc.vector.tensor_tensor(crs[2 * p_ + hh][:], esb[hh][:],
                                        rbY[hh][0:2, :], ALU.mult)
                if p_ == 0 and hh == 0:
                    nc.vector.tensor_tensor(m1acc[:], ysb[hh][:],
                                            rbY[hh][:], ALU.mult)
                else:
                    t_ = work.tile([P, NQ], F32, tag="nt", name="nt")
                    nc.vector.tensor_tensor(t_[:], ysb[hh][:],
                                            rbY[hh][:], ALU.mult)
                    eng = nc.gpsimd if p_ == 0 else nc.vector
                    eng.tensor_tensor(m1acc[:], m1acc[:], t_[:], ALU.add)

        # ---------------- MLP channel 128 + relu + W2 + residual ----------
        c4a = sing.tile([2, NQ], BF16, name="c4a")
        c4b = sing.tile([2, NQ], BF16, name="c4b")
        nc.vector.tensor_tensor(c4a[:], crs[0][:], crs[1][:], ALU.add)
        nc.vector.tensor_tensor(c4b[:], crs[2][:], crs[3][:], ALU.add)
        nc.vector.tensor_tensor(c4a[:], c4a[:], c4b[:], ALU.add)
        c4p = psE.tile([4, NQ], F32, tag="E", name="c4p")
        nc.tensor.matmul(c4p[0:1, :], ones2b[:], c4a[:], start=True,
                         stop=True)
        m1T = sing.tile([P, NQ], BF16, name="m1T")
        nc.scalar.activation(m1T[:], m1acc[:], ACTF.Relu, bias=biases[:, 0:1])
        m1L = sing.tile([1, NQ], BF16, name="m1L")
        # b1L has the 4.0 (sum of den*rinv over heads) pre-subtracted on host
        nc.scalar.activation(m1L[:], c4p[0:1, :], ACTF.Relu,
                             bias=biasesL[0:1, 0:1])
        pcf = psY.tile([P, NQ], F32, tag="Y", name="pcf")
        nc.tensor.matmul(pcf[:], w2a[:, 0:P], m1T[:], start=True, stop=False)
        nc.tensor.matmul(pcf[:], w2b[:, 0:P], m1L[:], start=False, stop=True)
        pcfL = psE.tile([2, NQ], F32, tag="E", name="pcfL")
        nc.tensor.matmul(pcfL[0:1, :], w2a[:, P:C], m1T[:], start=True,
                         stop=False)
        nc.tensor.matmul(pcfL[0:1, :], w2b[:, P:C], m1L[:], start=False,
                         stop=True)
        cf = sing.tile([P, NQ], BF16, name="cf")
        nc.vector.scalar_tensor_tensor(cf[:], pcf[:], biases[:, 1:2],
                                       combT[:, 0:NQ], op0=ALU.add,
                                       op1=ALU.add)
        cl_row = sing.tile([1, NQ], BF16, name="cl_row")
        nc.vector.scalar_tensor_tensor(cl_row[:], pcfL[0:1, :],
                                       biasesL[0:1, 1:2], cxr[0:1, 0:NQ],
                                       op0=ALU.add, op1=ALU.add)

        # ---------------- hypernetwork r|u (z-trick, transposed) ----------
        zq = [sing.tile([P, 4 * NQ], BF16, tag=f"zq{g}", name=f"zq{g}")
              for g in range(8)]

        # r | u: bias matmul first (no z dependency), then 32 d-planes,
        # then the c128-channel term via qcl.
        ones32 = sing.tile([1, QV], BF16, name="ones32")
        nc.gpsimd.memset(ones32[:], 1.0)
        repp = psS.tile([P, N], F32, tag="ps", name="repp")
        nc.tensor.matmul(repp[0:QV, 0:NQ], ones32[:], cl_row[:], start=True,
                         stop=True)
        for g in range(8):
            nc.vector.tensor_tensor(
                zq[g][:].rearrange("p (a b) -> p a b", b=NQ),
                cf[:, None, 0:NQ].to_broadcast((P, 4, NQ)),
                qb[:, g * 2048:(g + 1) * 2048].rearrange(
                    "p (a b) -> p a b", b=NQ),
                ALU.mult)
        qcl = sing.tile([QV, NQ], BF16, name="qcl")
        nc.vector.tensor_tensor(qcl[:], qvT[:], repp[0:QV, 0:NQ], ALU.mult)
        pru = psY.tile([P, NQ], F32, tag="Y", name="pru")
        nc.tensor.matmul(pru[:], smalls[:, P:256], qvT[:], start=True,
                         stop=False)
        for d in range(QV):
            g, dd = d // 4, d % 4
            nc.tensor.matmul(pru[:], wzru[:, d * P:(d + 1) * P],
                             zq[g][:, dd * NQ:(dd + 1) * NQ],
                             start=False, stop=False)
        nc.tensor.matmul(pru[:], smalls[:, 0:P], qcl[:], start=False,
                         stop=True)
        ru = sing.tile([P, NQ], BF16, name="ru")
        nc.scalar.activation(ru[:], pru[:], ACTF.Sigmoid)

        # ---------------- candidate gate (packed dynamic + host static) ---
        # selc2: rows 0:64 = r*hn, rows 64:128 = copy (for 2-plane packing)
        selc2 = sing.tile([P, NQ], BF16, name="selc2")
        nc.vector.tensor_tensor(selc2[0:D, :], ru[0:D, :], combT[0:D, 0:NQ],
                                ALU.mult)
        nc.vector.tensor_copy(selc2[D:P, :], selc2[0:D, :])
        u64 = sing.tile([D, NQ], BF16, name="u64")
        nc.scalar.copy(u64[:], ru[D:P, :])
        for g in range(4):
            nc.vector.tensor_tensor(
                zq[g][:].rearrange("p (a b) -> p a b", b=NQ),
                selc2[:, None, 0:NQ].to_broadcast((P, 4, NQ)),
                qb2[:, g * 2048:(g + 1) * 2048].rearrange(
                    "p (a b) -> p a b", b=NQ),
                ALU.mult)
        pc = psY.tile([P, NQ], F32, tag="Y", name="pc")
        for dd in range(16):
            g, di = dd // 4, dd % 4
            nc.tensor.matmul(pc[0:D, :], wzc2[:, dd * D:(dd + 1) * D],
                             zq[g][:, di * NQ:(di + 1) * NQ],
                             start=dd == 0, stop=dd == 15)
        pcs = sing.tile([D, NQ], BF16, name="pcs")
        nc.vector.tensor_tensor(pcs[:], pc[0:D, :], statc[:], ALU.add)
        cand = sing.tile([D, NQ], BF16, name="cand")
        nc.scalar.activation(cand[:], pcs[:], ACTF.Tanh)

        # out = hn + u*(cand - hn)   (hn = selc2 rows 0:64, u = ru 64:128)
        d1 = sing.tile([D, NQ], BF16, name="d1")
        nc.vector.tensor_tensor(d1[:], cand[:], selc2[0:D, :], ALU.subtract)
        nc.vector.tensor_tensor(d1[:], d1[:], u64[:], ALU.mult)
        outT = sing.tile([D, NQ], F32, name="outT")
        for j in range(4):
            nc.vector.tensor_tensor(outT[:, ts(j, P)], d1[:, ts(j, P)],
                                    selc2[0:D, ts(j, P)], ALU.add)
            pt = psE.tile([P, D], F32, tag="E", name="pt")
            nc.tensor.transpose(pt[:, 0:D], outT[:, ts(j, P)],
                                identf[0:D, 0:D])
            ob = work.tile([P, D], F32, tag="ob", name="ob")
            nc.vector.tensor_copy(ob[:], pt[:, 0:D])
            nc.sync.dma_start(out_d[ts(j, P), :], ob[:])

    return nc


_NC_CACHE = {}


def _get_nc():
    key = bool(USE_LRELU[0])
    if key not in _NC_CACHE:
        nc = build_graph(hw_leaky=key)
        if not nc.is_finalized():
            nc.finalize()
        _NC_CACHE[key] = nc
    return _NC_CACHE[key]


# channel reorder: new order = [h (64) | x (65)]
_R = np.concatenate([np.arange(65, 129), np.arange(0, 65)])
_BF = ml_dtypes.bfloat16
_E4 = ml_dtypes.float8_e4m3


def _bf(a):
    return np.ascontiguousarray(np.asarray(a, np.float32).astype(_BF))


def _f8(a, scale):
    x = np.clip(np.asarray(a, np.float32) * scale, -240, 240)
    return np.ascontiguousarray(x.astype(_E4))


def _prep_shared(Wq, bq, Wk, bk, Wv, bv, W1, b1, W2, b2, Wr, br, Wu, bu,
                 Wc, bc):
    f32 = np.float32
    Wq, bq = np.asarray(Wq, f32), np.asarray(bq, f32)
    Wk, bk = np.asarray(Wk, f32), np.asarray(bk, f32)
    Wv, bv = np.asarray(Wv, f32), np.asarray(bv, f32)
    W1, b1 = np.asarray(W1, f32).reshape(H, C, C), np.asarray(b1, f32)
    W2, b2 = np.asarray(W2, f32), np.asarray(b2, f32)

    # qk packed: per pair cols [k_h0|0|q_h0/4|0|k_h1|0|q_h1/4|0] (16 each);
    # biases ride the psum->sbuf copies as per-partition bias APs (bkq).
    Wq_r, Wk_r = Wq[:, _R, :], Wk[:, _R, :]
    kqw = np.zeros((C, 256), f32)
    bkq = np.zeros((KD, 2 * H), f32)
    for h in range(H):
        base = (h // 2) * 128 + (h % 2) * 64
        kqw[0:129, base:base + 16] = Wk_r[h]
        kqw[0:129, base + 32:base + 48] = Wq_r[h] * 0.25
        bkq[:, h] = bk[h]
        bkq[:, H + h] = bq[h] * 0.25
    kqw8 = np.zeros((P, 2, 256), f32)
    kqw8[:, 0, :] = kqw[0:128]
    kqw8[0, 1, :] = kqw[128]

    # V with W1 folded: U_h = Wv_h @ W1_h, contraction rows reordered
    U = np.stack([(Wv[h] @ W1[h])[_R] for h in range(H)])    # [H, 129, 129]
    wv1 = np.ascontiguousarray(
        np.transpose(U, (1, 0, 2)).reshape(C, H * C))        # [129, 516]
    b1_eff = b1 + sum(bv[h] @ W1[h] for h in range(H))
    wv18 = np.zeros((P, 2, 516), f32)
    wv18[:, 0, :] = wv1[0:128]
    wv18[0, 1, :] = wv1[128]

    w2r = W2[:, _R]                                          # cols reordered
    b2r = b2[_R]
    biases = np.zeros((C, 4), f32)
    biases[:, 0] = b1_eff
    biases[:, 1] = b2r
    # the c4 partition_all_reduce folds in sum_h den*rinv ~= H
    biases[128, 0] -= H

    Wr_r = np.asarray(Wr, f32)[:, _R, :]
    Wu_r = np.asarray(Wu, f32)[:, _R, :]
    Wc_r = np.asarray(Wc, f32)[:, _R, :]
    wzru = np.ascontiguousarray(np.transpose(
        np.concatenate([Wr_r[:, 0:128, :], Wu_r[:, 0:128, :]], 2),
        (1, 0, 2)).reshape(P, QV * P))
    # packed candidate weights [128, 16*64]
    wzc2 = np.zeros((P, 16, D), f32)
    for dd in range(16):
        wzc2[0:D, dd] = Wc_r[2 * dd, 0:D, :]
        wzc2[D:P, dd] = Wc_r[2 * dd + 1, 0:D, :]
    wzc2 = np.ascontiguousarray(wzc2.reshape(P, 16 * D))
    smalls = np.zeros((QV, 256), f32)
    smalls[:, 0:64] = Wr_r[:, 128, :]
    smalls[:, 64:128] = Wu_r[:, 128, :]
    smalls[:, 128:192] = np.asarray(br, f32)
    smalls[:, 192:256] = np.asarray(bu, f32)

    return dict(
        kqw8=_f8(kqw8.reshape(P, 512), 64.0),
        bkq=np.ascontiguousarray(bkq),
        wv18=_f8(wv18.reshape(P, 1032), 64.0),
        w2a=_bf(w2r[0:128]), w2b=_bf(w2r[128:129]),
        biases=np.ascontiguousarray(biases[0:128]),
        biasesL=np.ascontiguousarray(biases[128:129]),
        wzru=_bf(wzru), wzc2=_bf(wzc2), smalls=_bf(smalls),
        _Wc_r=Wc_r, _bc=np.asarray(bc, f32),
    )


def _prep_core(b, x, h, query_vectors, adj, nodes_n, shared):
    f32 = np.float32
    idx = nodes_n[b * NQ:(b + 1) * NQ].astype(np.int64)
    rest = np.setdiff1d(np.arange(N, dtype=np.int64), idx)
    perm = np.concatenate([idx, rest])
    comb = np.concatenate([x[b][perm], h[b][perm]], 1)[:, _R]  # [N,129] reord
    combT = np.ascontiguousarray(comb.T.astype(_BF))           # [129, N]
    comb8 = np.zeros((P, 2, N), f32)
    comb8[:, 0, :] = combT[0:128].astype(f32)
    comb8[0, 1, :] = combT[128].astype(f32)
    qv = query_vectors[b * NQ:(b + 1) * NQ]                    # [512, 32]
    qvT = np.ascontiguousarray(qv.T.astype(_BF))               # [32, 512]
    # qb [128, 32*512]: qb[p, d*512 + m] = qv[m, d]  (uint16 view = fast)
    qvT_u16 = qvT.view(np.uint16)
    qb = np.ascontiguousarray(
        np.broadcast_to(qvT_u16[None, :, :], (P, QV, NQ))
        .reshape(P, QV * NQ)).view(_BF)
    # qb2 [128, 16*512]: qb2[p, dd*512+m] = qv[m, 2dd + p//64]
    qb2 = np.empty((2, 16, NQ), np.uint16)
    for pl in range(2):
        qb2[pl] = qvT_u16[pl::2, :]
    qb2 = np.ascontiguousarray(
        np.broadcast_to(qb2[:, None, :, :], (2, D, 16, NQ))
        .reshape(P, 16 * NQ)).view(_BF)
    adjneg = np.where(adj[np.ix_(idx, perm)] != 0, 0.0, NEG).astype(f32).T
    adjN = np.ascontiguousarray(
        adjneg.reshape(NT, P, NQ).transpose(1, 0, 2).reshape(P, NT * NQ)
    ).astype(_BF)
    # host-computed static candidate preactivation:
    #   statc[o,m] = sum_{i in 64:129, d} Wc_r[d? ...] -- uses x rows + c128
    Wc_r, bc = shared["_Wc_r"], shared["_bc"]
    qvf = qvT.astype(f32)                                      # [32, 512]
    selS = combT[D:C, :NQ].astype(f32)                         # [65, 512]
    zS = (selS[:, None, :] * qvf[None, :, :]).reshape(65 * QV, NQ)
    WcS = Wc_r[:, D:C, :].transpose(1, 0, 2).reshape(65 * QV, D)
    statc = WcS.T @ zS + bc.T @ qvf                            # [64, 512]
    d = dict((k, v) for k, v in shared.items() if not k.startswith("_"))
    d["comb8"] = _f8(comb8.reshape(P, 2 * N), 8.0)
    d["combT"] = np.ascontiguousarray(combT[0:128])
    d["c128"] = np.ascontiguousarray(combT[128:129])
    d["qvT"] = qvT
    d["qb"] = qb
    d["qb2"] = qb2
    d["adjN"] = adjN
    d["statc"] = np.ascontiguousarray(statc.astype(f32))
    return d


def make_in_maps(x, h, query_vectors, adj, nodes_b, nodes_n, **weights):
    x = np.asarray(x, np.float32)
    h = np.asarray(h, np.float32)
    query_vectors = np.asarray(query_vectors, np.float32)
    adj = np.asarray(adj)
    nodes_n = np.asarray(nodes_n)
    shared = _prep_shared(**weights)
    return [_prep_core(b, x, h, query_vectors, adj, nodes_n, shared)
            for b in range(B)]


def kernel(x, h, query_vectors, adj, nodes_b, nodes_n,
           Wq, bq, Wk, bk, Wv, bv, W1, b1, W2, b2,
           Wr, br, Wu, bu, Wc, bc):
    in_maps = make_in_maps(
        x, h, query_vectors, adj, nodes_b, nodes_n,
        Wq=Wq, bq=bq, Wk=Wk, bk=bk, Wv=Wv, bv=bv, W1=W1, b1=b1, W2=W2, b2=b2,
        Wr=Wr, br=br, Wu=Wu, bu=bu, Wc=Wc, bc=bc)
    nc = _get_nc()
    res = run_bass_kernel_spmd(nc, in_maps, list(range(B)))
    outs = [np.asarray(res.results[b]["out"], np.float32) for b in range(B)]
    return np.concatenate(outs, axis=0)
